# revision 6
# baseline (speedup 1.0000x reference)
"""Trainium2 Bass kernel for nn_AbstractLiquidRecurrent (liquid time-constant RNN).

Model (reference):
    x0 = 0
    per time step t (T=256):   inp = i_t @ W_in.T + b    [B,N]
      per unfold k (K=6):      f = tanh(x @ W_rec.T + inp)
                               x = (x + dt_k*f*A) / (1 + dt_k*(1/tau + f))
    output: all x_t stacked -> [B, T, N]

Kernel strategy (8 NeuronCores, data-parallel over batch, 16 rows/core):
  - State kept TRANSPOSED: y.T as [n (4 chunks of 128 partitions), b=16 free],
    so the recurrent matmul is W-stationary / x-moving and never needs an
    in-loop transpose.
  - A is folded into the weights host-side (Wt = diag(A) @ W_rec.T, state
    y = x/A), which simplifies the elementwise update to
        y' = (y*R + f) / (R + 1/tau + f),   R = K/dt   (per batch, per t)
    and removes all per-partition-chunk coefficients from the hot loop.
  - Input projection accumulated into a PSUM bank once per t; copied (+bias)
    to SBUF; each unfold's matmul PSUM banks are PRELOADED with it via a
    ScalarE copy (has_written bits armed once at kernel start with dummy
    matmuls, so start=False matmuls accumulate on top of the preload).
  - tanh on ScalarE (single ACT table set), reciprocal via the custom DVE op
    RECIPROCAL_APPROX_FAST (tanh and reciprocal cannot share an ACT table).
  - 2-group software pipeline over the 4 n-chunks: group g's epilogue
    overlaps the other group's matmuls / next unfold's first matmuls.
  - Output y_t is DMA'd out per t in transposed layout; the host unshards,
    transposes back, and multiplies by A.
"""

import numpy as np

import concourse.bass as bass
import concourse.tile as tile
from concourse import bacc, mybir
from concourse.bass_utils import run_bass_kernel_spmd

# Problem constants (hardcoded per contract)
N = 512
F = 256
KUNF = 6
B, T = 128, 256
NCORES = 8
BLOC = B // NCORES          # 16 batch rows per core
NCH = N // 128              # 4 n-chunks
FCH = F // 128              # 2 f-chunks

f32 = mybir.dt.float32
bf16 = mybir.dt.bfloat16
f16 = mybir.dt.float16

# Matmul input dtype: "f32" (exact) or "bf16" (fast, full range, ~3 decimal
# digit weights/activations; PSUM accumulation stays fp32 either way).
MM_DTYPE = "f32"

# Reciprocal flavor: "fast" (1 DVE op, ~51 ULP), "accurate" (2 ops, ~2 ULP)
RECIP = "accurate"

_DT = {"f32": f32, "bf16": bf16, "f16": f16}
_NP = {"f32": np.float32, "bf16": None, "f16": np.float16}


def _np_cast(arr, mode):
    if mode == "f32":
        return np.ascontiguousarray(arr, dtype=np.float32)
    if mode == "bf16":
        import ml_dtypes
        return np.ascontiguousarray(arr.astype(ml_dtypes.bfloat16))
    if mode == "f16":
        return np.ascontiguousarray(arr, dtype=np.float16)
    raise ValueError(mode)


def build(t_run=T, mm_mode=MM_DTYPE):
    """Build the Bass module for one core (SPMD across 8)."""
    mdt = _DT[mm_mode]
    nc = bacc.Bacc("TRN2", target_bir_lowering=False, debug=False)

    # ---- DRAM I/O ----
    wrec_d = nc.dram_tensor("wrec", [128, NCH * NCH * 128], mdt, kind="ExternalInput").ap()
    win_d = nc.dram_tensor("win", [128, FCH * NCH * 128], mdt, kind="ExternalInput").ap()
    it_d = nc.dram_tensor("it", [128, t_run * FCH * BLOC], mdt, kind="ExternalInput").ap()
    rt_d = nc.dram_tensor("rt", [1, t_run * BLOC], f32, kind="ExternalInput").ap()
    invtau_d = nc.dram_tensor("invtau", [128, NCH], f32, kind="ExternalInput").ap()
    bvec_d = nc.dram_tensor("bvec", [128, NCH], f32, kind="ExternalInput").ap()
    yout_d = nc.dram_tensor("yout", [t_run, 128, NCH * BLOC], f32, kind="ExternalOutput").ap()

    W = NCH * BLOC   # 64: free width of the merged state tiles
    G = 2            # pipeline groups (2 n-chunks each)
    GW = W // G      # 32: free width per group

    with tile.TileContext(nc) as tc:
        import contextlib
        ctx = contextlib.ExitStack()
        with ctx:
            consts = ctx.enter_context(tc.tile_pool(name="consts", bufs=1))
            state = ctx.enter_context(tc.tile_pool(name="state", bufs=3))
            work = ctx.enter_context(tc.tile_pool(name="work", bufs=2))
            prep = ctx.enter_context(tc.tile_pool(name="prep", bufs=2))
            psum = ctx.enter_context(tc.tile_pool(name="psum", bufs=1, space="PSUM"))

            # ---- constant loads ----
            w_sb = consts.tile([128, NCH * NCH * 128], mdt)
            nc.sync.dma_start(w_sb[:], wrec_d[:])
            win_sb = consts.tile([128, FCH * NCH * 128], mdt)
            nc.sync.dma_start(win_sb[:], win_d[:])
            it_sb = consts.tile([128, t_run * FCH * BLOC], mdt)
            nc.sync.dma_start(it_sb[:], it_d[:])
            rt_sb = consts.tile([1, t_run * BLOC], f32)
            nc.sync.dma_start(rt_sb[:], rt_d[:])
            invtau_sb = consts.tile([128, NCH], f32)
            nc.sync.dma_start(invtau_sb[:], invtau_d[:])
            bvec_sb = consts.tile([128, NCH], f32)
            nc.sync.dma_start(bvec_sb[:], bvec_d[:])
            ones_sb = consts.tile([1, 128], f32)
            nc.vector.memset(ones_sb[:], 1.0)
            junk1 = consts.tile([1, GW], mdt)
            nc.vector.memset(junk1[:], 0.0)
            junk2 = consts.tile([1, 128], mdt)
            nc.vector.memset(junk2[:], 0.0)

            # persistent PSUM tiles: 2 z-groups, input projection, R broadcast
            zg = [psum.tile([128, GW], f32, name=f"zg{g}", tag=f"zg{g}") for g in range(G)]
            pin = psum.tile([128, W], f32, tag="pin")
            prt = psum.tile([128, BLOC], f32, tag="prt")

            # arm has_written bits of the z banks once (dummy matmuls)
            for g in range(G):
                nc.tensor.matmul(zg[g][:], lhsT=junk2[:], rhs=junk1[:],
                                 start=True, stop=True)

            # initial state y = 0, yR = 0
            y_cur = state.tile([128, W], f32, tag="y")
            nc.vector.memset(y_cur[:], 0.0)
            yr_cur = state.tile([128, W], f32, tag="yr")
            nc.vector.memset(yr_cur[:], 0.0)
            if mm_mode != "f32":
                ym_cur = state.tile([128, W], mdt, tag="ym")
                nc.vector.memset(ym_cur[:], 0.0)
            else:
                ym_cur = y_cur

            def w_tile(kc, mc):
                off = (kc * NCH + mc) * 128
                return w_sb[:, off:off + 128]

            def win_tile(fc, mc):
                off = (fc * NCH + mc) * 128
                return win_sb[:, off:off + 128]

            def prep_t(t):
                """Per-time-step prep: input projection, R tile, P2, inp+b."""
                # input projection for all 4 n-chunks into pin bank
                for mc in range(NCH):
                    for fc in range(FCH):
                        nc.tensor.matmul(
                            pin[:, mc * BLOC:(mc + 1) * BLOC],
                            lhsT=win_tile(fc, mc),
                            rhs=it_sb[:, (t * FCH + fc) * BLOC:(t * FCH + fc + 1) * BLOC],
                            start=(fc == 0), stop=(fc == FCH - 1),
                        )
                # R_t broadcast down the partitions via K=1 matmul
                nc.tensor.matmul(prt[:], lhsT=ones_sb[:],
                                 rhs=rt_sb[:, t * BLOC:(t + 1) * BLOC],
                                 start=True, stop=True)
                rtile = prep.tile([128, BLOC], f32, tag="rtile")
                nc.scalar.activation(rtile[:], prt[:],
                                     mybir.ActivationFunctionType.Copy)
                # P2 = R + 1/tau  (broadcast R along chunks, 1/tau along b)
                p2 = prep.tile([128, W], f32, tag="p2")
                nc.vector.tensor_add(
                    p2[:],
                    rtile[:].unsqueeze(1).broadcast_to([128, NCH, BLOC]),
                    invtau_sb[:].unsqueeze(2).broadcast_to([128, NCH, BLOC]),
                )
                # inp = (i_t @ W_in.T) + b
                inp = prep.tile([128, W], f32, tag="inp")
                nc.vector.tensor_add(
                    inp[:],
                    pin[:],
                    bvec_sb[:].unsqueeze(2).broadcast_to([128, NCH, BLOC]),
                )
                return rtile, p2, inp

            rtile, p2, inp = prep_t(0)
            # initial preload of z banks with inp(0)
            for g in range(G):
                nc.scalar.activation(zg[g][:], inp[:, g * GW:(g + 1) * GW],
                                     mybir.ActivationFunctionType.Copy)

            for t in range(t_run):
                nxt = None
                for k in range(KUNF):
                    last_unfold = (k == KUNF - 1)
                    # ---- matmuls: z += Wt.T@y, ordered kc-pairs first for
                    # pipelining with the previous epilogue ----
                    for kc_pair in ((0, 1), (2, 3)):
                        for mc in range(NCH):
                            g, sub = divmod(mc, NCH // G)
                            for kc in kc_pair:
                                nc.tensor.matmul(
                                    zg[g][:, sub * BLOC:(sub + 1) * BLOC],
                                    lhsT=w_tile(kc, mc),
                                    rhs=ym_cur[:, kc * BLOC:(kc + 1) * BLOC],
                                    start=False, stop=(kc == NCH - 1),
                                    skip_group_check=True,
                                )
                    # mid-t prep for t+1 (emitted once, during unfold 2)
                    if k == 2 and t + 1 < t_run:
                        nxt = prep_t(t + 1)

                    # ---- epilogue (per group) ----
                    y_new = state.tile([128, W], f32, tag="y")
                    if not last_unfold:
                        yr_new = state.tile([128, W], f32, name="yr_new", tag="yr")
                    else:
                        yr_new = None
                    if mm_mode != "f32":
                        ym_new = state.tile([128, W], mdt, name="ym_new", tag="ym")
                    else:
                        ym_new = y_new
                    f_t = work.tile([128, W], f32, tag="f")
                    d_t = work.tile([128, W], f32, tag="d")
                    rden_t = work.tile([128, W], f32, tag="rden")
                    nm_t = work.tile([128, W], f32, tag="nm")
                    rscr_t = work.tile([128, W], f32, tag="rscr")
                    for g in range(G):
                        sl = slice(g * GW, (g + 1) * GW)
                        nc.scalar.activation(f_t[:, sl], zg[g][:],
                                             mybir.ActivationFunctionType.Tanh)
                        # preload this group's bank for the NEXT unfold
                        src = inp if (not last_unfold or t + 1 >= t_run) else nxt[2]
                        nc.scalar.activation(zg[g][:], src[:, sl],
                                             mybir.ActivationFunctionType.Copy)
                        nc.vector.tensor_add(d_t[:, sl], f_t[:, sl], p2[:, sl])
                        if RECIP == "accurate":
                            nc.vector.reciprocal_approx_accurate(
                                out=rden_t[:, sl], in_=d_t[:, sl],
                                scratch=rscr_t[:, sl])
                        else:
                            nc.vector.reciprocal_approx_fast(out=rden_t[:, sl],
                                                             in_=d_t[:, sl])
                        nc.vector.tensor_add(nm_t[:, sl], f_t[:, sl], yr_cur[:, sl])
                        nc.vector.tensor_mul(y_new[:, sl], nm_t[:, sl], rden_t[:, sl])
                        if mm_mode != "f32":
                            nc.vector.tensor_copy(ym_new[:, sl], y_new[:, sl])
                        if not last_unfold:
                            # yR for the next unfold (same R_t)
                            nc.vector.tensor_mul(
                                yr_new[:, sl], y_new[:, sl],
                                rtile[:].unsqueeze(1).broadcast_to([128, NCH // G, BLOC]),
                            )
                    y_cur = y_new
                    ym_cur = ym_new
                    if not last_unfold:
                        yr_cur = yr_new

                # ---- end of time step: output + roll prep to t+1 ----
                nc.sync.dma_start(yout_d[t], y_cur[:])
                if t + 1 < t_run:
                    rtile, p2, inp = nxt
                    yr_cur = state.tile([128, W], f32, tag="yr")
                    nc.vector.tensor_mul(
                        yr_cur[:], y_cur[:],
                        rtile[:].unsqueeze(1).broadcast_to([128, NCH, BLOC]),
                    )

    nc.compile()
    return nc


def _host_prep(i, delta_t, W_rec, W_in, b, A, tau, t_run, mm_mode):
    """Shard + lay out the inputs for each core."""
    i = np.asarray(i, dtype=np.float32)
    delta_t = np.asarray(delta_t, dtype=np.float32)
    W_rec = np.asarray(W_rec, dtype=np.float32)
    W_in = np.asarray(W_in, dtype=np.float32)
    b = np.asarray(b, dtype=np.float32)
    A = np.asarray(A, dtype=np.float32)
    tau = np.asarray(tau, dtype=np.float32)

    # Wt[k, m] = A[k] * W_rec[m, k]  -> tiles [p, (kc*4+mc)*128 + m]
    Wt = (W_rec * A[None, :]).T
    wrec = _np_cast(
        Wt.reshape(NCH, 128, NCH, 128).transpose(1, 0, 2, 3).reshape(128, -1),
        mm_mode)
    win = _np_cast(
        W_in.T.reshape(FCH, 128, NCH, 128).transpose(1, 0, 2, 3).reshape(128, -1),
        mm_mode)
    invtau = np.ascontiguousarray((1.0 / tau).reshape(NCH, 128).T, dtype=np.float32)
    bvec = np.ascontiguousarray(b.reshape(NCH, 128).T, dtype=np.float32)

    in_maps = []
    for c in range(NCORES):
        bsl = slice(c * BLOC, (c + 1) * BLOC)
        ii = i[bsl, :t_run]                    # [16, t, 256]
        it = _np_cast(
            ii.reshape(BLOC, t_run, FCH, 128).transpose(3, 1, 2, 0).reshape(128, -1),
            mm_mode)
        rt = np.ascontiguousarray(
            (KUNF / np.maximum(delta_t[bsl, :t_run], 1e-30)).T.reshape(1, -1),
            dtype=np.float32)
        in_maps.append({
            "wrec": wrec, "win": win, "it": it, "rt": rt,
            "invtau": invtau, "bvec": bvec,
        })
    return in_maps


def _host_unshard(results, A, t_run):
    """results[c]["yout"]: [t, 128, 64] -> full x [B, T, N] (x = A*y)."""
    A = np.asarray(A, dtype=np.float32)
    out = np.empty((B, t_run, N), dtype=np.float32)
    for c in range(NCORES):
        y = results[c]["yout"]                       # [t, 128, 4*16]
        y = y.reshape(t_run, 128, NCH, BLOC)
        # x[b, t, n=c*128+p] = A[n] * y[t, p, nc, b]
        xc = y.transpose(3, 0, 2, 1).reshape(BLOC, t_run, N)
        out[c * BLOC:(c + 1) * BLOC] = xc * A[None, None, :]
    return out


_BUILD_CACHE = {}


def _get_built(t_run, mm_mode):
    key = (t_run, mm_mode)
    if key not in _BUILD_CACHE:
        _BUILD_CACHE[key] = build(t_run, mm_mode)
    return _BUILD_CACHE[key]


def run(i, delta_t, W_rec, W_in, b, A, tau, t_run=T, mm_mode=MM_DTYPE, **rb_kwargs):
    nc = _get_built(t_run, mm_mode)
    in_maps = _host_prep(i, delta_t, W_rec, W_in, b, A, tau, t_run, mm_mode)
    res = run_bass_kernel_spmd(nc, in_maps, list(range(NCORES)), **rb_kwargs)
    out = _host_unshard(res.results, A, t_run)
    return out, res


def kernel(i, delta_t, W_rec, W_in, b, A, tau):
    out, _ = run(i, delta_t, W_rec, W_in, b, A, tau)
    return out


# revision 8
# speedup vs baseline: 1.6858x; 1.6858x over previous
"""Trainium2 Bass kernel for nn_AbstractLiquidRecurrent (liquid time-constant RNN).

Model (reference):
    x0 = 0
    per time step t (T=256):   inp = i_t @ W_in.T + b    [B,N]
      per unfold k (K=6):      f = tanh(x @ W_rec.T + inp)
                               x = (x + dt_k*f*A) / (1 + dt_k*(1/tau + f))
    output: all x_t stacked -> [B, T, N]

Kernel strategy (8 NeuronCores, data-parallel over batch, 16 rows/core):
  - State kept TRANSPOSED: y.T as [n (4 chunks of 128 partitions), b=16 free],
    so the recurrent matmul is W-stationary / x-moving and never needs an
    in-loop transpose.
  - A is folded into the weights host-side (Wt = diag(A) @ W_rec.T, state
    y = x/A), which simplifies the elementwise update to
        y' = (y*R + f) / (R + 1/tau + f),   R = K/dt   (per batch, per t)
  - Matmul dtype modes:
      "f32"    exact fp32 matmuls (432 ns/tile on HW, weight-load bound)
      "bf16"   single bf16 pass (39 ns/tile); moving operand read as the
               high 16 bits of the fp32 state via a strided AP (free cast)
      "split6" 3-way bf16 decomposition of both W and y; the 6 dominant
               cross products give ~24-bit effective mantissas (fp32-grade)
               at 6x39=234 ns/tile -- ~2x faster than "f32"
  - Input projection accumulated into a PSUM bank once per t; each unfold's
    matmul PSUM banks are PRELOADED with it via a ScalarE copy (has_written
    bits armed once at kernel start, so start=False matmuls accumulate).
  - tanh on ScalarE; reciprocal via custom DVE Newton ops (tanh and
    reciprocal cannot share an ACT table set).
  - 2-group software pipeline over the 4 n-chunks; the next unfold's
    matmuls are ordered to consume group-0 state columns first.
  - Output y_t is DMA'd out per t in transposed layout; the host unshards,
    transposes back, and multiplies by A.
"""

import time as _time

import numpy as np

import concourse.bass as bass
import concourse.tile as tile
from concourse import bacc, mybir
from concourse.bass_utils import run_bass_kernel_spmd

# Problem constants (hardcoded per contract)
N = 512
F = 256
KUNF = 6
B, T = 128, 256
NCORES = 8
BLOC = B // NCORES          # 16 batch rows per core
NCH = N // 128              # 4 n-chunks
FCH = F // 128              # 2 f-chunks

f32 = mybir.dt.float32
bf16 = mybir.dt.bfloat16

MM_DTYPE = "split6"
RECIP = "accurate"          # "fast" (1 op, 51 ULP) | "accurate" (2 ops, 2 ULP)
VERBOSE = False


def _bf16_split(arr, terms):
    """Split fp32 array into `terms` bf16 arrays summing to ~arr."""
    import ml_dtypes
    out = []
    rem = np.asarray(arr, dtype=np.float32).copy()
    for _ in range(terms):
        h = rem.astype(ml_dtypes.bfloat16)
        out.append(np.ascontiguousarray(h))
        rem = rem - h.astype(np.float32)
    return out


def _hi_view(ap):
    """bf16 view of the high 16 bits of an f32 AP (truncated bf16 cast)."""
    p, n = ap.shape
    return ap.bitcast(bf16).rearrange("p (n two) -> p n two", two=2)[:, :, 1]


def build(t_run=T, mm_mode=MM_DTYPE):
    """Build + compile the Bass module for one core (SPMD across 8)."""
    t0 = _time.time()
    nc = bacc.Bacc("TRN2", target_bir_lowering=False, debug=False,
                   disable_frame_to_traceback=True)

    n_w = {"f32": 1, "bf16": 1, "split6": 3}[mm_mode]
    mdt = f32 if mm_mode == "f32" else bf16

    # ---- DRAM I/O ----
    w_d = [nc.dram_tensor(f"wrec{j}", [128, NCH * NCH * 128], mdt,
                          kind="ExternalInput").ap() for j in range(n_w)]
    win_d = [nc.dram_tensor(f"win{j}", [128, FCH * NCH * 128], mdt,
                            kind="ExternalInput").ap() for j in range(n_w)]
    it_d = [nc.dram_tensor(f"it{j}", [128, t_run * FCH * BLOC], mdt,
                           kind="ExternalInput").ap() for j in range(n_w)]
    rt_d = nc.dram_tensor("rt", [1, t_run * BLOC], f32, kind="ExternalInput").ap()
    invtau_d = nc.dram_tensor("invtau", [128, NCH], f32, kind="ExternalInput").ap()
    bvec_d = nc.dram_tensor("bvec", [128, NCH], f32, kind="ExternalInput").ap()
    yout_d = nc.dram_tensor("yout", [t_run, 128, NCH * BLOC], f32,
                            kind="ExternalOutput").ap()

    W = NCH * BLOC   # 64 free width of merged state tiles
    G = 2            # pipeline groups (2 n-chunks each)
    GW = W // G      # 32 free width per group

    with tile.TileContext(nc) as tc:
        import contextlib
        ctx = contextlib.ExitStack()
        with ctx:
            consts = ctx.enter_context(tc.tile_pool(name="consts", bufs=1))
            state = ctx.enter_context(tc.tile_pool(name="state", bufs=3))
            work = ctx.enter_context(tc.tile_pool(name="work", bufs=2))
            prep = ctx.enter_context(tc.tile_pool(name="prep", bufs=2))
            psum = ctx.enter_context(tc.tile_pool(name="psum", bufs=1, space="PSUM"))

            # ---- constant loads ----
            w_sb, win_sb, it_sb = [], [], []
            for j in range(n_w):
                wj = consts.tile([128, NCH * NCH * 128], mdt, name=f"w_sb{j}")
                nc.sync.dma_start(wj[:], w_d[j][:])
                w_sb.append(wj)
                winj = consts.tile([128, FCH * NCH * 128], mdt, name=f"win_sb{j}")
                nc.sync.dma_start(winj[:], win_d[j][:])
                win_sb.append(winj)
                itj = consts.tile([128, t_run * FCH * BLOC], mdt, name=f"it_sb{j}")
                nc.sync.dma_start(itj[:], it_d[j][:])
                it_sb.append(itj)
            rt_sb = consts.tile([1, t_run * BLOC], f32)
            nc.sync.dma_start(rt_sb[:], rt_d[:])
            invtau_sb = consts.tile([128, NCH], f32)
            nc.sync.dma_start(invtau_sb[:], invtau_d[:])
            bvec_sb = consts.tile([128, NCH], f32)
            nc.sync.dma_start(bvec_sb[:], bvec_d[:])
            ones_sb = consts.tile([1, 128], f32)
            nc.vector.memset(ones_sb[:], 1.0)
            junk1 = consts.tile([1, GW], mdt)
            nc.vector.memset(junk1[:], 0.0)
            junk2 = consts.tile([1, 128], mdt)
            nc.vector.memset(junk2[:], 0.0)

            # persistent PSUM tiles
            zg = [psum.tile([128, GW], f32, name=f"zg{g}", tag=f"zg{g}")
                  for g in range(G)]
            pin = psum.tile([128, W], f32, tag="pin")
            prt = psum.tile([128, BLOC], f32, tag="prt")

            # arm has_written bits of the z banks once
            for g in range(G):
                nc.tensor.matmul(zg[g][:], lhsT=junk2[:], rhs=junk1[:],
                                 start=True, stop=True)

            # initial state y = 0 (+ split terms), yR = 0
            y_cur = state.tile([128, W], f32, tag="y")
            nc.vector.memset(y_cur[:], 0.0)
            yr_cur = state.tile([128, W], f32, tag="yr")
            nc.vector.memset(yr_cur[:], 0.0)
            if mm_mode == "split6":
                y1_cur = state.tile([128, W], bf16, tag="y1")
                nc.vector.memset(y1_cur[:], 0.0)
                y2_cur = state.tile([128, W], bf16, tag="y2")
                nc.vector.memset(y2_cur[:], 0.0)
            else:
                y1_cur = y2_cur = None

            def w_tile(j, kc, mc):
                off = (kc * NCH + mc) * 128
                return w_sb[j][:, off:off + 128]

            def win_tile(j, fc, mc):
                off = (fc * NCH + mc) * 128
                return win_sb[j][:, off:off + 128]

            # matmul pass list: (w_index, y_source_index) ordered so that
            # later passes depend on later-computed y split terms.
            if mm_mode == "split6":
                mm_passes = [(0, 0), (1, 0), (0, 1), (2, 0), (1, 1), (0, 2)]
            else:
                mm_passes = [(0, 0)]

            def y_sources():
                """Current state's moving-operand sources per split index."""
                if mm_mode == "f32":
                    return [y_cur[:]]
                if mm_mode == "bf16":
                    return [_hi_view(y_cur[:])]
                return [_hi_view(y_cur[:]), y1_cur[:], y2_cur[:]]

            def prep_t(t):
                """Per-time-step prep: input projection, R tile, P2, inp+b."""
                # mc outer so each PSUM region's accumulation group is
                # contiguous (start=True clears has_written for the whole
                # bank, so a region must not be revisited after a later
                # start=True).
                seq = [(pj, sj, fc) for (pj, sj) in mm_passes
                       for fc in range(FCH)]
                for mc in range(NCH):
                    for idx, (pj, sj, fc) in enumerate(seq):
                        nc.tensor.matmul(
                            pin[:, mc * BLOC:(mc + 1) * BLOC],
                            lhsT=win_tile(pj, fc, mc),
                            rhs=it_sb[sj][:, (t * FCH + fc) * BLOC:
                                          (t * FCH + fc + 1) * BLOC],
                            start=(mc == 0 and idx == 0),
                            stop=(idx == len(seq) - 1),
                            skip_group_check=True,
                        )
                nc.tensor.matmul(prt[:], lhsT=ones_sb[:],
                                 rhs=rt_sb[:, t * BLOC:(t + 1) * BLOC],
                                 start=True, stop=True)
                rtile = prep.tile([128, BLOC], f32, tag="rtile")
                nc.scalar.activation(rtile[:], prt[:],
                                     mybir.ActivationFunctionType.Copy)
                p2 = prep.tile([128, W], f32, tag="p2")
                nc.vector.tensor_add(
                    p2[:],
                    rtile[:].unsqueeze(1).broadcast_to([128, NCH, BLOC]),
                    invtau_sb[:].unsqueeze(2).broadcast_to([128, NCH, BLOC]),
                )
                inp = prep.tile([128, W], f32, tag="inp")
                nc.vector.tensor_add(
                    inp[:],
                    pin[:],
                    bvec_sb[:].unsqueeze(2).broadcast_to([128, NCH, BLOC]),
                )
                return rtile, p2, inp

            rtile, p2, inp = prep_t(0)
            for g in range(G):
                nc.scalar.activation(zg[g][:], inp[:, g * GW:(g + 1) * GW],
                                     mybir.ActivationFunctionType.Copy)

            for t in range(t_run):
                nxt = None
                for k in range(KUNF):
                    last_unfold = (k == KUNF - 1)
                    ys = y_sources()
                    # ---- matmuls, kc-pair-major then pass-major ----
                    for kcp in ((0, 1), (2, 3)):
                        for pi, (pj, sj) in enumerate(mm_passes):
                            for mc in range(NCH):
                                g, sub = divmod(mc, NCH // G)
                                for kc in kcp:
                                    is_last = (kcp[1] == NCH - 1
                                               and pi == len(mm_passes) - 1
                                               and kc == kcp[1])
                                    nc.tensor.matmul(
                                        zg[g][:, sub * BLOC:(sub + 1) * BLOC],
                                        lhsT=w_tile(pj, kc, mc),
                                        rhs=ys[sj][:, kc * BLOC:(kc + 1) * BLOC],
                                        start=False, stop=is_last,
                                        skip_group_check=True,
                                    )
                    if k == 2 and t + 1 < t_run:
                        nxt = prep_t(t + 1)

                    # ---- epilogue (per group) ----
                    y_new = state.tile([128, W], f32, tag="y")
                    if not last_unfold:
                        yr_new = state.tile([128, W], f32, name="yr_new", tag="yr")
                    else:
                        yr_new = None
                    if mm_mode == "split6":
                        y1_new = state.tile([128, W], bf16, name="y1_new", tag="y1")
                        y2_new = state.tile([128, W], bf16, name="y2_new", tag="y2")
                    f_t = work.tile([128, W], f32, tag="f")
                    d_t = work.tile([128, W], f32, tag="d")
                    rden_t = work.tile([128, W], f32, tag="rden")
                    nm_t = work.tile([128, W], f32, tag="nm")
                    rscr_t = work.tile([128, W], f32, tag="rscr")
                    trem_t = work.tile([128, W], f32, tag="trem")
                    for g in range(G):
                        sl = slice(g * GW, (g + 1) * GW)
                        nc.scalar.activation(f_t[:, sl], zg[g][:],
                                             mybir.ActivationFunctionType.Tanh)
                        src = inp if (not last_unfold or t + 1 >= t_run) else nxt[2]
                        nc.scalar.activation(zg[g][:], src[:, sl],
                                             mybir.ActivationFunctionType.Copy)
                        nc.gpsimd.tensor_add(nm_t[:, sl], f_t[:, sl], yr_cur[:, sl])
                        nc.vector.tensor_add(d_t[:, sl], f_t[:, sl], p2[:, sl])
                        if RECIP == "accurate":
                            nc.vector.reciprocal_approx_accurate(
                                out=rden_t[:, sl], in_=d_t[:, sl],
                                scratch=rscr_t[:, sl])
                        else:
                            nc.vector.reciprocal_approx_fast(out=rden_t[:, sl],
                                                             in_=d_t[:, sl])
                        nc.vector.tensor_mul(y_new[:, sl], nm_t[:, sl],
                                             rden_t[:, sl])
                        if mm_mode == "split6":
                            nc.vector.tensor_sub(trem_t[:, sl], y_new[:, sl],
                                                 _hi_view(y_new[:, sl]))
                            nc.vector.tensor_copy(y1_new[:, sl], trem_t[:, sl])
                            nc.vector.tensor_sub(y2_new[:, sl], trem_t[:, sl],
                                                 y1_new[:, sl])
                        if not last_unfold:
                            nc.vector.tensor_mul(
                                yr_new[:, sl], y_new[:, sl],
                                rtile[:].unsqueeze(1).broadcast_to(
                                    [128, NCH // G, BLOC]),
                            )
                    y_cur = y_new
                    if mm_mode == "split6":
                        y1_cur, y2_cur = y1_new, y2_new
                    if not last_unfold:
                        yr_cur = yr_new

                # ---- end of time step: output + roll prep to t+1 ----
                nc.sync.dma_start(yout_d[t], y_cur[:])
                if t + 1 < t_run:
                    rtile, p2, inp = nxt
                    yr_cur = state.tile([128, W], f32, name="yr_roll", tag="yr")
                    nc.vector.tensor_mul(
                        yr_cur[:], y_cur[:],
                        rtile[:].unsqueeze(1).broadcast_to([128, NCH, BLOC]),
                    )

    t1 = _time.time()
    nc.compile()
    t2 = _time.time()
    if VERBOSE:
        print(f"[build] trace+schedule {t1-t0:.1f}s, bacc compile {t2-t1:.1f}s",
              flush=True)
    return nc


def _host_prep(i, delta_t, W_rec, W_in, b, A, tau, t_run, mm_mode):
    """Shard + lay out the inputs for each core."""
    i = np.asarray(i, dtype=np.float32)
    delta_t = np.asarray(delta_t, dtype=np.float32)
    W_rec = np.asarray(W_rec, dtype=np.float32)
    W_in = np.asarray(W_in, dtype=np.float32)
    b = np.asarray(b, dtype=np.float32)
    A = np.asarray(A, dtype=np.float32)
    tau = np.asarray(tau, dtype=np.float32)

    n_w = {"f32": 1, "bf16": 1, "split6": 3}[mm_mode]

    def tiles_rec(m):   # [512,512] (k, m) -> [128, 16*128]
        return m.reshape(NCH, 128, NCH, 128).transpose(1, 0, 2, 3).reshape(128, -1)

    def tiles_in(m):    # [256,512] (k, m) -> [128, 8*128]
        return m.reshape(FCH, 128, NCH, 128).transpose(1, 0, 2, 3).reshape(128, -1)

    Wt = (W_rec * A[None, :]).T          # Wt[k, m] = A[k] * W_rec[m, k]
    WinT = W_in.T
    if mm_mode == "f32":
        w_arrs = [np.ascontiguousarray(tiles_rec(Wt), dtype=np.float32)]
        win_arrs = [np.ascontiguousarray(tiles_in(WinT), dtype=np.float32)]
    else:
        w_arrs = [tiles_rec(x.astype(np.float32)).astype(x.dtype)
                  for x in _bf16_split(Wt, n_w)]
        win_arrs = [tiles_in(x.astype(np.float32)).astype(x.dtype)
                    for x in _bf16_split(WinT, n_w)]
        w_arrs = [np.ascontiguousarray(x) for x in w_arrs]
        win_arrs = [np.ascontiguousarray(x) for x in win_arrs]

    invtau = np.ascontiguousarray((1.0 / tau).reshape(NCH, 128).T, dtype=np.float32)
    bvec = np.ascontiguousarray(b.reshape(NCH, 128).T, dtype=np.float32)

    in_maps = []
    for c in range(NCORES):
        bsl = slice(c * BLOC, (c + 1) * BLOC)
        ii = i[bsl, :t_run]                    # [16, t, 256]
        def tile_i(x):
            return (x.reshape(BLOC, t_run, FCH, 128)
                    .transpose(3, 1, 2, 0).reshape(128, -1))
        if mm_mode == "f32":
            it_arrs = [np.ascontiguousarray(tile_i(ii), dtype=np.float32)]
        else:
            it_arrs = [np.ascontiguousarray(tile_i(x.astype(np.float32)).astype(x.dtype))
                       for x in _bf16_split(ii, n_w)]
        rt = np.ascontiguousarray(
            (KUNF / np.maximum(delta_t[bsl, :t_run], 1e-30)).T.reshape(1, -1),
            dtype=np.float32)
        m = {"rt": rt, "invtau": invtau, "bvec": bvec}
        for j in range(n_w):
            m[f"wrec{j}"] = w_arrs[j]
            m[f"win{j}"] = win_arrs[j]
            m[f"it{j}"] = it_arrs[j]
        in_maps.append(m)
    return in_maps


def _host_unshard(results, A, t_run):
    """results[c]["yout"]: [t, 128, 64] -> full x [B, T, N] (x = A*y)."""
    A = np.asarray(A, dtype=np.float32)
    out = np.empty((B, t_run, N), dtype=np.float32)
    for c in range(NCORES):
        y = results[c]["yout"].reshape(t_run, 128, NCH, BLOC)
        xc = y.transpose(3, 0, 2, 1).reshape(BLOC, t_run, N)
        out[c * BLOC:(c + 1) * BLOC] = xc * A[None, None, :]
    return out


_BUILD_CACHE = {}


def _get_built(t_run, mm_mode):
    key = (t_run, mm_mode)
    if key not in _BUILD_CACHE:
        _BUILD_CACHE[key] = build(t_run, mm_mode)
    return _BUILD_CACHE[key]


def run(i, delta_t, W_rec, W_in, b, A, tau, t_run=T, mm_mode=MM_DTYPE, **rb_kwargs):
    nc = _get_built(t_run, mm_mode)
    in_maps = _host_prep(i, delta_t, W_rec, W_in, b, A, tau, t_run, mm_mode)
    res = run_bass_kernel_spmd(nc, in_maps, list(range(NCORES)), **rb_kwargs)
    out = _host_unshard(res.results, A, t_run)
    return out, res


def kernel(i, delta_t, W_rec, W_in, b, A, tau):
    out, _ = run(i, delta_t, W_rec, W_in, b, A, tau)
    return out


# revision 10
# speedup vs baseline: 1.7226x; 1.0218x over previous
"""Trainium2 Bass kernel for nn_AbstractLiquidRecurrent (liquid time-constant RNN).

Model (reference):
    x0 = 0
    per time step t (T=256):   inp = i_t @ W_in.T + b    [B,N]
      per unfold k (K=6):      f = tanh(x @ W_rec.T + inp)
                               x = (x + dt_k*f*A) / (1 + dt_k*(1/tau + f))
    output: all x_t stacked -> [B, T, N]

Kernel strategy (8 NeuronCores, data-parallel over batch, 16 rows/core):
  - State kept TRANSPOSED: y.T as [n (4 chunks of 128 partitions), b=16 free],
    so the recurrent matmul is W-stationary / x-moving and never needs an
    in-loop transpose.
  - A is folded into the weights host-side (Wt = diag(A) @ W_rec.T, state
    y = x/A), which simplifies the elementwise update to
        y' = (y*R + f) / (R + 1/tau + f),   R = K/dt   (per batch, per t)
  - Matmul dtype modes:
      "f32"    exact fp32 matmuls (432 ns/tile on HW, weight-load bound)
      "bf16"   single bf16 pass (39 ns/tile); moving operand read as the
               high 16 bits of the fp32 state via a strided AP (free cast)
      "split6" 3-way bf16 decomposition of both W and y; the 6 dominant
               cross products give ~24-bit effective mantissas (fp32-grade)
               at 6x39=234 ns/tile -- ~2x faster than "f32"
  - Input projection accumulated into a PSUM bank once per t; each unfold's
    matmul PSUM banks are PRELOADED with it via a ScalarE copy (has_written
    bits armed once at kernel start, so start=False matmuls accumulate).
  - tanh on ScalarE; reciprocal via custom DVE Newton ops (tanh and
    reciprocal cannot share an ACT table set).
  - 2-group software pipeline over the 4 n-chunks; the next unfold's
    matmuls are ordered to consume group-0 state columns first.
  - Output y_t is DMA'd out per t in transposed layout; the host unshards,
    transposes back, and multiplies by A.
"""

import time as _time

import numpy as np

import concourse.bass as bass
import concourse.tile as tile
from concourse import bacc, mybir
from concourse.bass_utils import run_bass_kernel_spmd

# Problem constants (hardcoded per contract)
N = 512
F = 256
KUNF = 6
B, T = 128, 256
NCORES = 8
BLOC = B // NCORES          # 16 batch rows per core
NCH = N // 128              # 4 n-chunks
FCH = F // 128              # 2 f-chunks

f32 = mybir.dt.float32
bf16 = mybir.dt.bfloat16

MM_DTYPE = "split6"
RECIP = "accurate"          # "fast" (1 op, 51 ULP) | "accurate" (2 ops, 2 ULP)
VERBOSE = False


def _bf16_split(arr, terms):
    """Split fp32 array into `terms` bf16 arrays summing to ~arr."""
    import ml_dtypes
    out = []
    rem = np.asarray(arr, dtype=np.float32).copy()
    for _ in range(terms):
        h = rem.astype(ml_dtypes.bfloat16)
        out.append(np.ascontiguousarray(h))
        rem = rem - h.astype(np.float32)
    return out


def _hi_view(ap):
    """bf16 view of the high 16 bits of an f32 AP (truncated bf16 cast)."""
    p, n = ap.shape
    return ap.bitcast(bf16).rearrange("p (n two) -> p n two", two=2)[:, :, 1]


def build(t_run=T, mm_mode=MM_DTYPE):
    """Build + compile the Bass module for one core (SPMD across 8)."""
    t0 = _time.time()
    nc = bacc.Bacc("TRN2", target_bir_lowering=False, debug=False,
                   disable_frame_to_traceback=True)

    n_w = {"f32": 1, "bf16": 1, "split6": 3}[mm_mode]
    mdt = f32 if mm_mode == "f32" else bf16

    # ---- DRAM I/O ----
    w_d = [nc.dram_tensor(f"wrec{j}", [128, NCH * NCH * 128], mdt,
                          kind="ExternalInput").ap() for j in range(n_w)]
    win_d = [nc.dram_tensor(f"win{j}", [128, FCH * NCH * 128], mdt,
                            kind="ExternalInput").ap() for j in range(n_w)]
    it_d = [nc.dram_tensor(f"it{j}", [128, t_run * FCH * BLOC], mdt,
                           kind="ExternalInput").ap() for j in range(n_w)]
    rt_d = nc.dram_tensor("rt", [1, t_run * BLOC], f32, kind="ExternalInput").ap()
    invtau_d = nc.dram_tensor("invtau", [128, NCH], f32, kind="ExternalInput").ap()
    bvec_d = nc.dram_tensor("bvec", [128, NCH], f32, kind="ExternalInput").ap()
    yout_d = nc.dram_tensor("yout", [t_run, 128, NCH * BLOC], f32,
                            kind="ExternalOutput").ap()

    W = NCH * BLOC   # 64 free width of merged state tiles
    G = 2            # pipeline groups (2 n-chunks each)
    GW = W // G      # 32 free width per group

    with tile.TileContext(nc) as tc:
        import contextlib
        ctx = contextlib.ExitStack()
        with ctx:
            consts = ctx.enter_context(tc.tile_pool(name="consts", bufs=1))
            state = ctx.enter_context(tc.tile_pool(name="state", bufs=3))
            work = ctx.enter_context(tc.tile_pool(name="work", bufs=2))
            prep = ctx.enter_context(tc.tile_pool(name="prep", bufs=2))
            psum = ctx.enter_context(tc.tile_pool(name="psum", bufs=1, space="PSUM"))

            # ---- constant loads ----
            w_sb, win_sb, it_sb = [], [], []
            for j in range(n_w):
                wj = consts.tile([128, NCH * NCH * 128], mdt, name=f"w_sb{j}")
                nc.sync.dma_start(wj[:], w_d[j][:])
                w_sb.append(wj)
                winj = consts.tile([128, FCH * NCH * 128], mdt, name=f"win_sb{j}")
                nc.sync.dma_start(winj[:], win_d[j][:])
                win_sb.append(winj)
                itj = consts.tile([128, t_run * FCH * BLOC], mdt, name=f"it_sb{j}")
                nc.sync.dma_start(itj[:], it_d[j][:])
                it_sb.append(itj)
            rt_sb = consts.tile([1, t_run * BLOC], f32)
            nc.sync.dma_start(rt_sb[:], rt_d[:])
            invtau_sb = consts.tile([128, NCH], f32)
            nc.sync.dma_start(invtau_sb[:], invtau_d[:])
            bvec_sb = consts.tile([128, NCH], f32)
            nc.sync.dma_start(bvec_sb[:], bvec_d[:])
            ones_sb = consts.tile([1, 128], f32)
            nc.vector.memset(ones_sb[:], 1.0)
            junk1 = consts.tile([1, GW], mdt)
            nc.vector.memset(junk1[:], 0.0)
            junk2 = consts.tile([1, 128], mdt)
            nc.vector.memset(junk2[:], 0.0)

            # persistent PSUM tiles
            zg = [psum.tile([128, GW], f32, name=f"zg{g}", tag=f"zg{g}")
                  for g in range(G)]
            pin = psum.tile([128, W], f32, tag="pin")
            prt = psum.tile([128, BLOC], f32, tag="prt")

            # arm has_written bits of the z banks once
            for g in range(G):
                nc.tensor.matmul(zg[g][:], lhsT=junk2[:], rhs=junk1[:],
                                 start=True, stop=True)

            # initial state y = 0 (+ split terms), yR = 0
            y_cur = state.tile([128, W], f32, tag="y")
            nc.vector.memset(y_cur[:], 0.0)
            yr_cur = state.tile([128, W], f32, tag="yr")
            nc.vector.memset(yr_cur[:], 0.0)
            if mm_mode == "split6":
                y1_cur = state.tile([128, W], bf16, tag="y1")
                nc.vector.memset(y1_cur[:], 0.0)
                y2_cur = state.tile([128, W], bf16, tag="y2")
                nc.vector.memset(y2_cur[:], 0.0)
            else:
                y1_cur = y2_cur = None

            def w_tile(j, kc, mc):
                off = (kc * NCH + mc) * 128
                return w_sb[j][:, off:off + 128]

            def win_tile(j, fc, mc):
                off = (fc * NCH + mc) * 128
                return win_sb[j][:, off:off + 128]

            # matmul pass list: (w_index, y_source_index) ordered so that
            # later passes depend on later-computed y split terms.
            if mm_mode == "split6":
                mm_passes = [(0, 0), (1, 0), (0, 1), (2, 0), (1, 1), (0, 2)]
            else:
                mm_passes = [(0, 0)]

            def y_sources():
                """Current state's moving-operand sources per split index."""
                if mm_mode == "f32":
                    return [y_cur[:]]
                if mm_mode == "bf16":
                    return [_hi_view(y_cur[:])]
                return [_hi_view(y_cur[:]), y1_cur[:], y2_cur[:]]

            def prep_t(t):
                """Per-time-step prep: input projection, R tile, P2, inp+b."""
                # mc outer so each PSUM region's accumulation group is
                # contiguous (start=True clears has_written for the whole
                # bank, so a region must not be revisited after a later
                # start=True).
                seq = [(pj, sj, fc) for (pj, sj) in mm_passes
                       for fc in range(FCH)]
                for mc in range(NCH):
                    for idx, (pj, sj, fc) in enumerate(seq):
                        nc.tensor.matmul(
                            pin[:, mc * BLOC:(mc + 1) * BLOC],
                            lhsT=win_tile(pj, fc, mc),
                            rhs=it_sb[sj][:, (t * FCH + fc) * BLOC:
                                          (t * FCH + fc + 1) * BLOC],
                            start=(mc == 0 and idx == 0),
                            stop=(idx == len(seq) - 1),
                            skip_group_check=True,
                        )
                nc.tensor.matmul(prt[:], lhsT=ones_sb[:],
                                 rhs=rt_sb[:, t * BLOC:(t + 1) * BLOC],
                                 start=True, stop=True)
                rtile = prep.tile([128, BLOC], f32, tag="rtile")
                nc.scalar.activation(rtile[:], prt[:],
                                     mybir.ActivationFunctionType.Copy)
                p2 = prep.tile([128, W], f32, tag="p2")
                nc.vector.tensor_add(
                    p2[:],
                    rtile[:].unsqueeze(1).broadcast_to([128, NCH, BLOC]),
                    invtau_sb[:].unsqueeze(2).broadcast_to([128, NCH, BLOC]),
                )
                inp = prep.tile([128, W], f32, tag="inp")
                nc.vector.tensor_add(
                    inp[:],
                    pin[:],
                    bvec_sb[:].unsqueeze(2).broadcast_to([128, NCH, BLOC]),
                )
                return rtile, p2, inp

            rtile, p2, inp = prep_t(0)
            for g in range(G):
                nc.scalar.activation(zg[g][:], inp[:, g * GW:(g + 1) * GW],
                                     mybir.ActivationFunctionType.Copy)

            for t in range(t_run):
                nxt = None
                for k in range(KUNF):
                    last_unfold = (k == KUNF - 1)
                    ys = y_sources()
                    # fresh tiles for this unfold
                    y_new = state.tile([128, W], f32, tag="y")
                    if not last_unfold:
                        yr_new = state.tile([128, W], f32, name="yr_new", tag="yr")
                    else:
                        yr_new = None
                    if mm_mode == "split6":
                        y1_new = state.tile([128, W], bf16, name="y1_new", tag="y1")
                        y2_new = state.tile([128, W], bf16, name="y2_new", tag="y2")
                    f_t = work.tile([128, W], f32, tag="f")
                    d_t = work.tile([128, W], f32, tag="d")
                    rden_t = work.tile([128, W], f32, tag="rden")
                    nm_t = work.tile([128, W], f32, tag="nm")
                    rscr_t = work.tile([128, W], f32, tag="rscr")
                    trem_t = work.tile([128, W], f32, tag="trem")

                    # bank-major: all of bank g's matmuls, then its epilogue
                    # (overlapping the other bank's matmuls on the PE).
                    for g in range(G):
                        mcs = range(g * (NCH // G), (g + 1) * (NCH // G))
                        cnt = 0
                        total = len(mm_passes) * (NCH // G) * NCH
                        for kcp in ((0, 1), (2, 3)):
                            for pj, sj in mm_passes:
                                for mc in mcs:
                                    sub = mc % (NCH // G)
                                    for kc in kcp:
                                        cnt += 1
                                        nc.tensor.matmul(
                                            zg[g][:, sub * BLOC:(sub + 1) * BLOC],
                                            lhsT=w_tile(pj, kc, mc),
                                            rhs=ys[sj][:, kc * BLOC:(kc + 1) * BLOC],
                                            start=False, stop=(cnt == total),
                                            skip_group_check=True,
                                        )
                        # ---- epilogue for bank g ----
                        sl = slice(g * GW, (g + 1) * GW)
                        nc.scalar.activation(f_t[:, sl], zg[g][:],
                                             mybir.ActivationFunctionType.Tanh)
                        src = inp if (not last_unfold or t + 1 >= t_run) else nxt[2]
                        nc.scalar.activation(zg[g][:], src[:, sl],
                                             mybir.ActivationFunctionType.Copy)
                        nc.vector.tensor_add(d_t[:, sl], f_t[:, sl], p2[:, sl])
                        nc.vector.tensor_add(nm_t[:, sl], f_t[:, sl], yr_cur[:, sl])
                        if RECIP == "accurate":
                            nc.vector.reciprocal_approx_accurate(
                                out=rden_t[:, sl], in_=d_t[:, sl],
                                scratch=rscr_t[:, sl])
                        else:
                            nc.vector.reciprocal_approx_fast(out=rden_t[:, sl],
                                                             in_=d_t[:, sl])
                        nc.vector.tensor_mul(y_new[:, sl], nm_t[:, sl],
                                             rden_t[:, sl])
                        if mm_mode == "split6":
                            nc.vector.tensor_sub(trem_t[:, sl], y_new[:, sl],
                                                 _hi_view(y_new[:, sl]))
                            nc.vector.tensor_copy(y1_new[:, sl], trem_t[:, sl])
                            nc.vector.tensor_sub(y2_new[:, sl], trem_t[:, sl],
                                                 y1_new[:, sl])
                        if not last_unfold:
                            nc.vector.tensor_mul(
                                yr_new[:, sl], y_new[:, sl],
                                rtile[:].unsqueeze(1).broadcast_to(
                                    [128, NCH // G, BLOC]),
                            )
                        # mid-unfold prep for t+1 after bank 0 of unfold 2
                        if g == 0 and k == 2 and t + 1 < t_run:
                            nxt = prep_t(t + 1)
                    y_cur = y_new
                    if mm_mode == "split6":
                        y1_cur, y2_cur = y1_new, y2_new
                    if not last_unfold:
                        yr_cur = yr_new

                # ---- end of time step: output + roll prep to t+1 ----
                nc.sync.dma_start(yout_d[t], y_cur[:])
                if t + 1 < t_run:
                    rtile, p2, inp = nxt
                    yr_cur = state.tile([128, W], f32, name="yr_roll", tag="yr")
                    nc.vector.tensor_mul(
                        yr_cur[:], y_cur[:],
                        rtile[:].unsqueeze(1).broadcast_to([128, NCH, BLOC]),
                    )

    t1 = _time.time()
    nc.compile()
    t2 = _time.time()
    if VERBOSE:
        print(f"[build] trace+schedule {t1-t0:.1f}s, bacc compile {t2-t1:.1f}s",
              flush=True)
    return nc


def _host_prep(i, delta_t, W_rec, W_in, b, A, tau, t_run, mm_mode):
    """Shard + lay out the inputs for each core."""
    i = np.asarray(i, dtype=np.float32)
    delta_t = np.asarray(delta_t, dtype=np.float32)
    W_rec = np.asarray(W_rec, dtype=np.float32)
    W_in = np.asarray(W_in, dtype=np.float32)
    b = np.asarray(b, dtype=np.float32)
    A = np.asarray(A, dtype=np.float32)
    tau = np.asarray(tau, dtype=np.float32)

    n_w = {"f32": 1, "bf16": 1, "split6": 3}[mm_mode]

    def tiles_rec(m):   # [512,512] (k, m) -> [128, 16*128]
        return m.reshape(NCH, 128, NCH, 128).transpose(1, 0, 2, 3).reshape(128, -1)

    def tiles_in(m):    # [256,512] (k, m) -> [128, 8*128]
        return m.reshape(FCH, 128, NCH, 128).transpose(1, 0, 2, 3).reshape(128, -1)

    Wt = (W_rec * A[None, :]).T          # Wt[k, m] = A[k] * W_rec[m, k]
    WinT = W_in.T
    if mm_mode == "f32":
        w_arrs = [np.ascontiguousarray(tiles_rec(Wt), dtype=np.float32)]
        win_arrs = [np.ascontiguousarray(tiles_in(WinT), dtype=np.float32)]
    else:
        w_arrs = [tiles_rec(x.astype(np.float32)).astype(x.dtype)
                  for x in _bf16_split(Wt, n_w)]
        win_arrs = [tiles_in(x.astype(np.float32)).astype(x.dtype)
                    for x in _bf16_split(WinT, n_w)]
        w_arrs = [np.ascontiguousarray(x) for x in w_arrs]
        win_arrs = [np.ascontiguousarray(x) for x in win_arrs]

    invtau = np.ascontiguousarray((1.0 / tau).reshape(NCH, 128).T, dtype=np.float32)
    bvec = np.ascontiguousarray(b.reshape(NCH, 128).T, dtype=np.float32)

    in_maps = []
    for c in range(NCORES):
        bsl = slice(c * BLOC, (c + 1) * BLOC)
        ii = i[bsl, :t_run]                    # [16, t, 256]
        def tile_i(x):
            return (x.reshape(BLOC, t_run, FCH, 128)
                    .transpose(3, 1, 2, 0).reshape(128, -1))
        if mm_mode == "f32":
            it_arrs = [np.ascontiguousarray(tile_i(ii), dtype=np.float32)]
        else:
            it_arrs = [np.ascontiguousarray(tile_i(x.astype(np.float32)).astype(x.dtype))
                       for x in _bf16_split(ii, n_w)]
        rt = np.ascontiguousarray(
            (KUNF / np.maximum(delta_t[bsl, :t_run], 1e-30)).T.reshape(1, -1),
            dtype=np.float32)
        m = {"rt": rt, "invtau": invtau, "bvec": bvec}
        for j in range(n_w):
            m[f"wrec{j}"] = w_arrs[j]
            m[f"win{j}"] = win_arrs[j]
            m[f"it{j}"] = it_arrs[j]
        in_maps.append(m)
    return in_maps


def _host_unshard(results, A, t_run):
    """results[c]["yout"]: [t, 128, 64] -> full x [B, T, N] (x = A*y)."""
    A = np.asarray(A, dtype=np.float32)
    out = np.empty((B, t_run, N), dtype=np.float32)
    for c in range(NCORES):
        y = results[c]["yout"].reshape(t_run, 128, NCH, BLOC)
        xc = y.transpose(3, 0, 2, 1).reshape(BLOC, t_run, N)
        out[c * BLOC:(c + 1) * BLOC] = xc * A[None, None, :]
    return out


_BUILD_CACHE = {}


def _get_built(t_run, mm_mode):
    key = (t_run, mm_mode)
    if key not in _BUILD_CACHE:
        _BUILD_CACHE[key] = build(t_run, mm_mode)
    return _BUILD_CACHE[key]


def run(i, delta_t, W_rec, W_in, b, A, tau, t_run=T, mm_mode=MM_DTYPE, **rb_kwargs):
    nc = _get_built(t_run, mm_mode)
    in_maps = _host_prep(i, delta_t, W_rec, W_in, b, A, tau, t_run, mm_mode)
    res = run_bass_kernel_spmd(nc, in_maps, list(range(NCORES)), **rb_kwargs)
    out = _host_unshard(res.results, A, t_run)
    return out, res


def kernel(i, delta_t, W_rec, W_in, b, A, tau):
    out, _ = run(i, delta_t, W_rec, W_in, b, A, tau)
    return out
